# revision 3
# baseline (speedup 1.0000x reference)
"""Trainium2 Bass kernel for nn_DeepTransitionRNN_31928786878509.

kernel(**inputs) -> np.ndarray, matching reference.reference semantics:
a deep-transition GRU over T=512 steps, B=128 (packed-sequence masking),
D=H=256, L=4 transition layers.

Strategy: data-parallel over batch (16 rows/core on 8 cores). Each core runs
the full 512-step recurrence with h resident in SBUF in transposed layout
h^T [H-chunk partitions x batch]. The per-step serial chain is minimized:

  sub-layer blend  h' = gate_a * nn + (1 - gate_a) * h  is computed by ONE
  fused tensor_tensor_scan over triplet-interleaved tiles
     A = [tanh_out, gate_a, 0],  B = [w_or_0, p, 1],  init = 1
     j0: s = tanh + w            (folds the cell's  "+ l*(x@Wt)"  add)
     j1: s = gate_a*s + p        (p = gate_b*h, computed OFF the chain)
     j2: s = 0*s + 1             (re-arm)
  so the critical chain per sub-layer is mm -> sigmoid -> mult -> tanh -> scan.

Both signs of the z-gates are needed (z and 1-z); the weight pack stores Wz
and -Wz (and Tz / -Tz) so a single packed SIGMOID yields both. Off-chain work
(p-mults, gate copies into the scan operand slots, the per-step output DMA)
goes to the otherwise idle GpSimd engine. x-projections for step t+1 are
issued into the tensor queue during step t so only h-dependent matmuls sit on
the chain. Output is DMAed in transposed layout and unpacked on the host;
packed-sequence masking (out=0 for t >= lengths[b]) is applied on the host.
"""

import os
import numpy as np
from contextlib import ExitStack

import concourse.bass as bass
import concourse.bacc as bacc
import concourse.mybir as mybir
import concourse.tile as tile
from concourse.bass_utils import run_bass_kernel_spmd

f16 = mybir.dt.float16
f32 = mybir.dt.float32
AF = mybir.ActivationFunctionType
OP = mybir.AluOpType

T, B, D, H, L = 512, 128, 256, 256, 4
NCORE = 8
BS = B // NCORE
KC_D = D // 128
KC_H = H // 128
MC = H // 128
# chunk counts: Wr 4x2, Wl 4x2, Wz 4x2, nWz 4x2, Cx 2x2, Wt 2x2, Ch 2x2,
# per layer: Tr 2x2, Tz 2x2, nTz 2x2, Tn 2x2
NCH = (4 + 4 + 4 + 4) * MC + 3 * KC_H * MC + 4 * L * KC_H * MC  # 108

LAST_EXEC_NS = None  # set by kernel() when tracing is enabled


def _pack_weights(Wr, Wz, Wl, Wt, Cx, Ch, Tr, Tz, Tn):
    chunks = []

    def add(M):
        for kc in range(M.shape[0] // 128):
            for mc in range(MC):
                chunks.append(M[kc * 128:(kc + 1) * 128, mc * 128:(mc + 1) * 128])

    Wz = np.asarray(Wz)
    add(Wr); add(Wl); add(Wz); add(-Wz)
    add(Cx); add(Wt); add(Ch)
    for i in range(L):
        Tzi = np.asarray(Tz[i])
        add(Tr[i]); add(Tzi); add(-Tzi); add(Tn[i])
    arr = np.stack([np.asarray(c, dtype=np.float32) for c in chunks])
    arr = np.ascontiguousarray(arr.transpose(1, 0, 2).astype(np.float16))
    return arr


def _pack_x_shard(x_shard):
    Tn_ = x_shard.shape[0]
    y = np.asarray(x_shard, dtype=np.float16).reshape(Tn_, BS, KC_D, 128)
    y = y.transpose(0, 3, 2, 1)
    return np.ascontiguousarray(y.reshape(Tn_, 128, KC_D * BS))


def _build_nc(Tsteps):
    nc = bacc.Bacc(None, target_bir_lowering=False, debug=False)
    xin = nc.dram_tensor('xt', [Tsteps, 128, KC_D * BS], f16, kind='ExternalInput')
    win = nc.dram_tensor('wp', [128, NCH, 128], f16, kind='ExternalInput')
    oul = nc.dram_tensor('out', [Tsteps, 128, KC_H * BS], f16, kind='ExternalOutput')

    idx = {}
    pos = 0

    def reg(name, kt):
        nonlocal pos
        idx[name] = [[pos + kc * MC + mc for mc in range(MC)] for kc in range(kt)]
        pos += kt * MC

    reg('Wr', 4); reg('Wl', 4); reg('Wz', 4); reg('nWz', 4)
    reg('Cx', 2); reg('Wt', 2); reg('Ch', 2)
    for i in range(L):
        reg(f'Tr{i}', 2); reg(f'Tz{i}', 2); reg(f'nTz{i}', 2); reg(f'Tn{i}', 2)
    assert pos == NCH

    XBLK = 64  # steps per x-prefetch DMA

    with ExitStack() as ctx:
        tc = ctx.enter_context(tile.TileContext(nc))
        wpool = ctx.enter_context(tc.tile_pool(name='w', bufs=1))
        hpool = ctx.enter_context(tc.tile_pool(name='h', bufs=1))
        xpool = ctx.enter_context(tc.tile_pool(name='x', bufs=1))
        spool = ctx.enter_context(tc.tile_pool(name='s', bufs=2))
        # all PSUM pools single-buffered: every tile is read early in its
        # own sub-layer, so the next accumulation epoch never stalls
        ps_r_pool = ctx.enter_context(tc.tile_pool(name='ps_r', bufs=1, space='PSUM'))
        ps_lzz_pool = ctx.enter_context(tc.tile_pool(name='ps_lzz', bufs=1, space='PSUM'))
        ps_aux_pool = ctx.enter_context(tc.tile_pool(name='ps_aux', bufs=1, space='PSUM'))
        ps_rr_pool = ctx.enter_context(tc.tile_pool(name='ps_rr', bufs=1, space='PSUM'))
        ps_zz_pool = ctx.enter_context(tc.tile_pool(name='ps_zz', bufs=1, space='PSUM'))
        ps_tn_pool = ctx.enter_context(tc.tile_pool(name='ps_tn', bufs=1, space='PSUM'))

        W = wpool.tile([128, NCH, 128], f16)
        nc.gpsimd.dma_start(W[:], win[:])

        # whole input resident in SBUF (32 KiB/partition), loaded in 8 DMAs
        xbig = xpool.tile([128, Tsteps, KC_D * BS], f16)
        for c in range(Tsteps // XBLK):
            nc.sync.dma_start(
                xbig[:, c * XBLK:(c + 1) * XBLK],
                xin[c * XBLK:(c + 1) * XBLK].rearrange('t p f -> p t f'))

        HTa = hpool.tile([128, KC_H, BS, 3], f16, tag='hta')
        HTb = hpool.tile([128, KC_H, BS, 3], f16, tag='htb')
        nc.gpsimd.memset(HTa[:], 0.0)
        nc.gpsimd.memset(HTb[:], 0.0)
        A = hpool.tile([128, KC_H, BS, 3], f16, tag='scan_a')
        Bc = hpool.tile([128, KC_H, BS, 3], f16, tag='scan_bc')
        Bt = hpool.tile([128, KC_H, BS, 3], f16, tag='scan_bt')
        nc.gpsimd.memset(A[:], 0.0)
        nc.gpsimd.memset(Bc[:], 0.0)
        nc.gpsimd.memset(Bt[:], 0.0)
        nc.gpsimd.memset(Bc[:, :, :, 2], 1.0)
        nc.gpsimd.memset(Bt[:, :, :, 2], 1.0)

        def flat(ap):
            return ap.rearrange('p c b j -> p (c b j)')

        def mm(out_ap, name, kc, mc, rhs, start, stop):
            nc.tensor.matmul(out_ap, W[:, idx[name][kc][mc], :], rhs,
                             start=start, stop=stop)

        def h_ap(tile_, kc):
            return tile_[:, kc, :, 1]

        def xt_ap(t):
            return xbig[:, t].rearrange('p (c b) -> p c b', c=KC_D)

        xstash = {}

        def emit_xmms(t):
            """Allocate step-t cell PSUM tiles and emit the x-dependent
            matmuls (issued one step early so they fill tensor idle time)."""
            ps_r = ps_r_pool.tile([128, MC, BS], f32, tag='ps_r')
            ps_lzz = ps_lzz_pool.tile([128, 3, MC, BS], f32, tag='ps_lzz')
            ps_aux = ps_aux_pool.tile([128, 3, MC, BS], f32, tag='ps_aux')
            xt = xt_ap(t)
            for mc in range(MC):
                for kc in range(KC_D):
                    mm(ps_r[:, mc], 'Wr', kc, mc, xt[:, kc],
                       mc == 0 and kc == 0, False)
            for gi, g in ((0, 'Wl'), (1, 'Wz'), (2, 'nWz')):
                for mc in range(MC):
                    for kc in range(KC_D):
                        mm(ps_lzz[:, gi, mc], g, kc, mc, xt[:, kc],
                           gi == 0 and mc == 0 and kc == 0, False)
            for gi, g in ((1, 'Cx'), (2, 'Wt')):
                for mc in range(MC):
                    for kc in range(KC_D):
                        mm(ps_aux[:, gi, mc], g, kc, mc, xt[:, kc],
                           gi == 1 and mc == 0 and kc == 0, False)
            xstash[t] = (ps_r, ps_lzz, ps_aux)

        def emit_step(t, hcur, hother):
            hseq = [hcur if s % 2 == 0 else hother for s in range(2 + L)]
            h0 = hseq[0]
            ps_r, ps_lzz, ps_aux = xstash.pop(t)

            # --- cell: h-dependent matmuls (r first: it gates the chain) ---
            for mc in range(MC):
                for kc in range(KC_H):
                    mm(ps_r[:, mc], 'Wr', KC_D + kc, mc, h_ap(h0, kc), False,
                       mc == MC - 1 and kc == KC_H - 1)
            for gi, g in ((0, 'Wl'), (1, 'Wz'), (2, 'nWz')):
                for mc in range(MC):
                    for kc in range(KC_H):
                        mm(ps_lzz[:, gi, mc], g, KC_D + kc, mc, h_ap(h0, kc),
                           False, gi == 2 and mc == MC - 1 and kc == KC_H - 1)
            for mc in range(MC):
                for kc in range(KC_H):
                    mm(ps_aux[:, 0, mc], 'Ch', kc, mc, h_ap(h0, kc), False,
                       mc == MC - 1 and kc == KC_H - 1)
            if t + 1 < Tsteps:
                emit_xmms(t + 1)

            # --- cell elementwise ---
            s_r = spool.tile([128, MC, BS], f16, tag='s_r')
            nc.scalar.activation(s_r[:], ps_r[:], AF.Sigmoid)
            s3 = spool.tile([128, 3, MC, BS], f16, tag='s3')  # [sig_l, z, q]
            nc.scalar.activation(s3[:], ps_lzz[:], AF.Sigmoid)
            u = spool.tile([128, MC, BS], f16, tag='u')
            nc.vector.tensor_tensor(u[:], s_r[:], ps_aux[:, 0], OP.mult)
            v = spool.tile([128, MC, BS], f16, tag='v')
            nc.vector.tensor_tensor(v[:], u[:], ps_aux[:, 1], OP.add)
            nc.gpsimd.tensor_copy(A[:, :, :, 1], s3[:, 1])          # z
            # w reads PSUM, which GpSimd cannot access -> DVE (off-chain,
            # fills the tanh window)
            nc.vector.tensor_tensor(Bc[:, :, :, 0], s3[:, 0], ps_aux[:, 2],
                                    OP.mult)                        # w = l*(x@Wt)
            nc.gpsimd.tensor_tensor(Bc[:, :, :, 1], s3[:, 2], h0[:, :, :, 1],
                                    OP.mult)                        # p = q*h
            nc.scalar.activation(A[:, :, :, 0], v[:], AF.Tanh)
            nc.vector.tensor_tensor_scan(
                flat(hseq[1][:]), flat(A[:]), flat(Bc[:]), 1.0, OP.mult, OP.add)

            # --- transition layers ---
            for li in range(L):
                hp = hseq[1 + li]
                ps_rr = ps_rr_pool.tile([128, MC, BS], f32, tag='ps_rr')
                ps_zz = ps_zz_pool.tile([128, 2, MC, BS], f32, tag='ps_zz')
                ps_tn = ps_tn_pool.tile([128, MC, BS], f32, tag='ps_tn')
                for mc in range(MC):
                    for kc in range(KC_H):
                        mm(ps_rr[:, mc], f'Tr{li}', kc, mc, h_ap(hp, kc),
                           mc == 0 and kc == 0, mc == MC - 1 and kc == KC_H - 1)
                for gi, g in ((0, f'Tz{li}'), (1, f'nTz{li}')):
                    for mc in range(MC):
                        for kc in range(KC_H):
                            mm(ps_zz[:, gi, mc], g, kc, mc, h_ap(hp, kc),
                               gi == 0 and mc == 0 and kc == 0,
                               gi == 1 and mc == MC - 1 and kc == KC_H - 1)
                for mc in range(MC):
                    for kc in range(KC_H):
                        mm(ps_tn[:, mc], f'Tn{li}', kc, mc, h_ap(hp, kc),
                           mc == 0 and kc == 0, mc == MC - 1 and kc == KC_H - 1)
                s_rr = spool.tile([128, MC, BS], f16, tag='s_rr')
                nc.scalar.activation(s_rr[:], ps_rr[:], AF.Sigmoid)
                s_zq = spool.tile([128, 2, MC, BS], f16, tag='s_zq')  # [z, q]
                nc.scalar.activation(s_zq[:], ps_zz[:], AF.Sigmoid)
                m = spool.tile([128, MC, BS], f16, tag='m')
                nc.vector.tensor_tensor(m[:], s_rr[:], ps_tn[:], OP.mult)
                nc.gpsimd.tensor_copy(A[:, :, :, 1], s_zq[:, 1])     # q
                nc.gpsimd.tensor_tensor(Bt[:, :, :, 1], s_zq[:, 0],
                                        hp[:, :, :, 1], OP.mult)     # p = z*h
                nc.scalar.activation(A[:, :, :, 0], m[:], AF.Tanh)
                nc.vector.tensor_tensor_scan(
                    flat(hseq[2 + li][:]), flat(A[:]), flat(Bt[:]), 1.0,
                    OP.mult, OP.add)

            hf = hseq[1 + L]
            nc.gpsimd.dma_start(
                oul[t].rearrange('p (c b) -> p c b', c=KC_H), hf[:, :, :, 1])
            return hf

        emit_xmms(0)
        hcur, hother = HTa, HTb
        for t in range(Tsteps):
            emit_step(t, hcur, hother)
            hcur, hother = hother, hcur

    nc.compile()
    return nc


def _install_ntff_hook_shim():
    """The agent image lacks ``antenv.axon_hooks``; recreate it and register
    trn_boot's ctypes NTFF hook so trace=True works. Returns True on
    success."""
    import sys
    import types
    try:
        import antenv.axon_hooks  # noqa: F401
        return True
    except ImportError:
        pass
    try:
        import antenv
        from trn_agent_boot.trn_boot import _ntff_profile_via_ctypes
        mod = types.ModuleType('antenv.axon_hooks')
        mod._hook = _ntff_profile_via_ctypes('/opt/axon/libaxon_pjrt.so')
        mod.get_axon_ntff_profile_hook = lambda: mod._hook
        mod.set_axon_ntff_profile_hook = lambda h: setattr(mod, '_hook', h)
        sys.modules['antenv.axon_hooks'] = mod
        antenv.axon_hooks = mod
        return True
    except Exception as e:  # degrade to no-trace
        print(f'ntff hook shim failed: {e}')
        return False


def kernel(x, lengths, Wr, Wz, Wl, Wt, Cx, Ch, Tr, Tz, Tn):
    global LAST_EXEC_NS
    x = np.asarray(x)
    lengths = np.asarray(lengths)

    wp = _pack_weights(Wr, Wz, Wl, Wt, Cx, Ch, Tr, Tz, Tn)
    nc = _build_nc(T)

    in_maps = []
    for k in range(NCORE):
        xs = x[:, k * BS:(k + 1) * BS, :]
        in_maps.append({'xt': _pack_x_shard(xs), 'wp': wp})

    trace = bool(int(os.environ.get('RNN_KERNEL_TRACE', '0')))
    if trace:
        trace = _install_ntff_hook_shim()
    res = run_bass_kernel_spmd(nc, in_maps, core_ids=list(range(NCORE)),
                               trace=trace)
    LAST_EXEC_NS = res.exec_time_ns

    out = np.empty((T, B, H), np.float32)
    for k in range(NCORE):
        y = np.asarray(res.results[k]['out'], np.float32)  # [T,128,KC_H*BS]
        y = y.reshape(T, 128, KC_H, BS).transpose(0, 3, 2, 1)  # [T,BS,KC_H,128]
        out[:, k * BS:(k + 1) * BS, :] = y.reshape(T, BS, H)
    mask = np.arange(T)[:, None] < lengths[None, :]
    out *= mask[:, :, None].astype(np.float32)
    return out


# revision 4
# speedup vs baseline: 4.4702x; 4.4702x over previous
"""Trainium2 Bass kernel for nn_DeepTransitionRNN_31928786878509.

kernel(**inputs) -> np.ndarray, matching reference.reference semantics:
a deep-transition GRU over T=512 steps, B=128 (packed-sequence masking),
D=H=256, L=4 transition layers.

Strategy: data-parallel over batch (16 rows/core on 8 cores). Each core runs
the full 512-step recurrence with h resident in SBUF in transposed layout
h^T [H-chunk partitions x batch]. The per-step serial chain is minimized:

  sub-layer blend  h' = gate_a * nn + (1 - gate_a) * h  is computed by ONE
  fused tensor_tensor_scan over triplet-interleaved tiles
     A = [tanh_out, gate_a, 0],  B = [w_or_0, p, 1],  init = 1
     j0: s = tanh + w            (folds the cell's  "+ l*(x@Wt)"  add)
     j1: s = gate_a*s + p        (p = gate_b*h, computed OFF the chain)
     j2: s = 0*s + 1             (re-arm)
  so the critical chain per sub-layer is mm -> sigmoid -> mult -> tanh -> scan.

Both signs of the z-gates are needed (z and 1-z); the weight pack stores Wz
and -Wz (and Tz / -Tz) so a single packed SIGMOID yields both. Off-chain work
(p-mults, gate copies into the scan operand slots, the per-step output DMA)
goes to the otherwise idle GpSimd engine. x-projections for step t+1 are
issued into the tensor queue during step t so only h-dependent matmuls sit on
the chain. Output is DMAed in transposed layout and unpacked on the host;
packed-sequence masking (out=0 for t >= lengths[b]) is applied on the host.
"""

import os
import numpy as np
from contextlib import ExitStack

import concourse.bass as bass
import concourse.bacc as bacc
import concourse.mybir as mybir
import concourse.tile as tile
from concourse.bass_utils import run_bass_kernel_spmd

f16 = mybir.dt.float16
f32 = mybir.dt.float32
AF = mybir.ActivationFunctionType
OP = mybir.AluOpType

T, B, D, H, L = 512, 128, 256, 256, 4
NCORE = 8
BS = B // NCORE
KC_D = D // 128
KC_H = H // 128
MC = H // 128
# chunk counts: Wr 4x2, Wl 4x2, Wz 4x2, nWz 4x2, Cx 2x2, Wt 2x2, Ch 2x2,
# per layer: Tr 2x2, Tz 2x2, nTz 2x2, Tn 2x2
NCH = (4 + 4 + 4 + 4) * MC + 3 * KC_H * MC + 4 * L * KC_H * MC  # 108

LAST_EXEC_NS = None  # set by kernel() when tracing is enabled


def _pack_weights(Wr, Wz, Wl, Wt, Cx, Ch, Tr, Tz, Tn):
    chunks = []

    def add(M):
        for kc in range(M.shape[0] // 128):
            for mc in range(MC):
                chunks.append(M[kc * 128:(kc + 1) * 128, mc * 128:(mc + 1) * 128])

    Wz = np.asarray(Wz)
    add(Wr); add(Wl); add(Wz); add(-Wz)
    add(Cx); add(Wt); add(Ch)
    for i in range(L):
        Tzi = np.asarray(Tz[i])
        add(Tr[i]); add(Tzi); add(-Tzi); add(Tn[i])
    arr = np.stack([np.asarray(c, dtype=np.float32) for c in chunks])
    arr = np.ascontiguousarray(arr.transpose(1, 0, 2).astype(np.float16))
    return arr


def _pack_x_shard(x_shard):
    Tn_ = x_shard.shape[0]
    y = np.asarray(x_shard, dtype=np.float16).reshape(Tn_, BS, KC_D, 128)
    y = y.transpose(0, 3, 2, 1)
    return np.ascontiguousarray(y.reshape(Tn_, 128, KC_D * BS))


def _build_nc(Tsteps):
    nc = bacc.Bacc(None, target_bir_lowering=False, debug=False)
    xin = nc.dram_tensor('xt', [Tsteps, 128, KC_D * BS], f16, kind='ExternalInput')
    win = nc.dram_tensor('wp', [128, NCH, 128], f16, kind='ExternalInput')
    oul = nc.dram_tensor('out', [Tsteps, 128, KC_H * BS * 3], f16, kind='ExternalOutput')

    idx = {}
    pos = 0

    def reg(name, kt):
        nonlocal pos
        idx[name] = [[pos + kc * MC + mc for mc in range(MC)] for kc in range(kt)]
        pos += kt * MC

    reg('Wr', 4); reg('Wl', 4); reg('Wz', 4); reg('nWz', 4)
    reg('Cx', 2); reg('Wt', 2); reg('Ch', 2)
    for i in range(L):
        reg(f'Tr{i}', 2); reg(f'Tz{i}', 2); reg(f'nTz{i}', 2); reg(f'Tn{i}', 2)
    assert pos == NCH

    XBLK = 64  # steps per x-prefetch DMA

    with ExitStack() as ctx:
        tc = ctx.enter_context(tile.TileContext(nc))
        wpool = ctx.enter_context(tc.tile_pool(name='w', bufs=1))
        hpool = ctx.enter_context(tc.tile_pool(name='h', bufs=1))
        xpool = ctx.enter_context(tc.tile_pool(name='x', bufs=1))
        spool = ctx.enter_context(tc.tile_pool(name='s', bufs=2))
        # all PSUM pools single-buffered: every tile is read early in its
        # own sub-layer, so the next accumulation epoch never stalls
        ps_r_pool = ctx.enter_context(tc.tile_pool(name='ps_r', bufs=1, space='PSUM'))
        ps_lzz_pool = ctx.enter_context(tc.tile_pool(name='ps_lzz', bufs=1, space='PSUM'))
        ps_aux_pool = ctx.enter_context(tc.tile_pool(name='ps_aux', bufs=1, space='PSUM'))
        ps_rr_pool = ctx.enter_context(tc.tile_pool(name='ps_rr', bufs=1, space='PSUM'))
        ps_zz_pool = ctx.enter_context(tc.tile_pool(name='ps_zz', bufs=1, space='PSUM'))
        ps_tn_pool = ctx.enter_context(tc.tile_pool(name='ps_tn', bufs=1, space='PSUM'))

        W = wpool.tile([128, NCH, 128], f16)
        nc.gpsimd.dma_start(W[:], win[:])

        # whole input resident in SBUF (32 KiB/partition), loaded in 8 DMAs
        xbig = xpool.tile([128, Tsteps, KC_D * BS], f16)
        for c in range(Tsteps // XBLK):
            nc.sync.dma_start(
                xbig[:, c * XBLK:(c + 1) * XBLK],
                xin[c * XBLK:(c + 1) * XBLK].rearrange('t p f -> p t f'))

        HTa = hpool.tile([128, KC_H, BS, 3], f16, tag='hta')
        HTb = hpool.tile([128, KC_H, BS, 3], f16, tag='htb')
        nc.gpsimd.memset(HTa[:], 0.0)
        nc.gpsimd.memset(HTb[:], 0.0)
        A = hpool.tile([128, KC_H, BS, 3], f16, tag='scan_a')
        Bc = hpool.tile([128, KC_H, BS, 3], f16, tag='scan_bc')
        Bt = hpool.tile([128, KC_H, BS, 3], f16, tag='scan_bt')
        nc.gpsimd.memset(A[:], 0.0)
        nc.gpsimd.memset(Bc[:], 0.0)
        nc.gpsimd.memset(Bt[:], 0.0)
        nc.gpsimd.memset(Bc[:, :, :, 2], 1.0)
        nc.gpsimd.memset(Bt[:, :, :, 2], 1.0)

        def flat(ap):
            return ap.rearrange('p c b j -> p (c b j)')

        def mm(out_ap, name, kc, mc, rhs, start, stop):
            nc.tensor.matmul(out_ap, W[:, idx[name][kc][mc], :], rhs,
                             start=start, stop=stop)

        def h_ap(tile_, kc):
            return tile_[:, kc, :, 1]

        def xt_ap(t):
            return xbig[:, t].rearrange('p (c b) -> p c b', c=KC_D)

        xstash = {}

        def emit_xmms(t):
            """Allocate step-t cell PSUM tiles and emit the x-dependent
            matmuls (issued one step early so they fill tensor idle time)."""
            ps_r = ps_r_pool.tile([128, MC, BS], f32, tag='ps_r')
            ps_lzz = ps_lzz_pool.tile([128, 3, MC, BS], f32, tag='ps_lzz')
            ps_aux = ps_aux_pool.tile([128, 3, MC, BS], f32, tag='ps_aux')
            xt = xt_ap(t)
            for mc in range(MC):
                for kc in range(KC_D):
                    mm(ps_r[:, mc], 'Wr', kc, mc, xt[:, kc],
                       mc == 0 and kc == 0, False)
            for gi, g in ((0, 'Wl'), (1, 'nWz'), (2, 'Wz')):
                for mc in range(MC):
                    for kc in range(KC_D):
                        mm(ps_lzz[:, gi, mc], g, kc, mc, xt[:, kc],
                           gi == 0 and mc == 0 and kc == 0, False)
            for gi, g in ((1, 'Cx'), (2, 'Wt')):
                for mc in range(MC):
                    for kc in range(KC_D):
                        mm(ps_aux[:, gi, mc], g, kc, mc, xt[:, kc],
                           gi == 1 and mc == 0 and kc == 0, False)
            xstash[t] = (ps_r, ps_lzz, ps_aux)

        def emit_step(t, hcur, hother):
            hseq = [hcur if s % 2 == 0 else hother for s in range(2 + L)]
            h0 = hseq[0]
            ps_r, ps_lzz, ps_aux = xstash.pop(t)

            # --- cell: h-dependent matmuls (r first: it gates the chain) ---
            for mc in range(MC):
                for kc in range(KC_H):
                    mm(ps_r[:, mc], 'Wr', KC_D + kc, mc, h_ap(h0, kc), False,
                       mc == MC - 1 and kc == KC_H - 1)
            for gi, g in ((0, 'Wl'), (1, 'nWz'), (2, 'Wz')):
                for mc in range(MC):
                    for kc in range(KC_H):
                        mm(ps_lzz[:, gi, mc], g, KC_D + kc, mc, h_ap(h0, kc),
                           False, gi == 2 and mc == MC - 1 and kc == KC_H - 1)
            for mc in range(MC):
                for kc in range(KC_H):
                    mm(ps_aux[:, 0, mc], 'Ch', kc, mc, h_ap(h0, kc), False,
                       mc == MC - 1 and kc == KC_H - 1)
            if t + 1 < Tsteps:
                emit_xmms(t + 1)

            # --- cell elementwise ---
            s_r = spool.tile([128, MC, BS], f16, tag='s_r')
            nc.scalar.activation(s_r[:], ps_r[:], AF.Sigmoid)
            s3 = spool.tile([128, 2, MC, BS], f16, tag='s3')  # [sig_l, q]
            nc.scalar.activation(s3[:], ps_lzz[:, 0:2], AF.Sigmoid)
            nc.scalar.activation(A[:, :, :, 1], ps_lzz[:, 2], AF.Sigmoid)  # z
            u = spool.tile([128, MC, BS], f16, tag='u')
            nc.vector.tensor_tensor(u[:], s_r[:], ps_aux[:, 0], OP.mult)
            v = spool.tile([128, MC, BS], f16, tag='v')
            nc.vector.tensor_tensor(v[:], u[:], ps_aux[:, 1], OP.add)
            nc.vector.tensor_tensor(Bc[:, :, :, 0], s3[:, 0], ps_aux[:, 2],
                                    OP.mult)                        # w = l*(x@Wt)
            nc.vector.tensor_tensor(Bc[:, :, :, 1], s3[:, 1], h0[:, :, :, 1],
                                    OP.mult)                        # p = q*h
            nc.scalar.activation(A[:, :, :, 0], v[:], AF.Tanh)
            nc.vector.tensor_tensor_scan(
                flat(hseq[1][:]), flat(A[:]), flat(Bc[:]), 1.0, OP.mult, OP.add)

            # --- transition layers ---
            for li in range(L):
                hp = hseq[1 + li]
                ps_rr = ps_rr_pool.tile([128, MC, BS], f32, tag='ps_rr')
                ps_zz = ps_zz_pool.tile([128, 2, MC, BS], f32, tag='ps_zz')
                ps_tn = ps_tn_pool.tile([128, MC, BS], f32, tag='ps_tn')
                for mc in range(MC):
                    for kc in range(KC_H):
                        mm(ps_rr[:, mc], f'Tr{li}', kc, mc, h_ap(hp, kc),
                           mc == 0 and kc == 0, mc == MC - 1 and kc == KC_H - 1)
                for gi, g in ((0, f'Tz{li}'), (1, f'nTz{li}')):
                    for mc in range(MC):
                        for kc in range(KC_H):
                            mm(ps_zz[:, gi, mc], g, kc, mc, h_ap(hp, kc),
                               gi == 0 and mc == 0 and kc == 0,
                               gi == 1 and mc == MC - 1 and kc == KC_H - 1)
                for mc in range(MC):
                    for kc in range(KC_H):
                        mm(ps_tn[:, mc], f'Tn{li}', kc, mc, h_ap(hp, kc),
                           mc == 0 and kc == 0, mc == MC - 1 and kc == KC_H - 1)
                s_rr = spool.tile([128, MC, BS], f16, tag='s_rr')
                nc.scalar.activation(s_rr[:], ps_rr[:], AF.Sigmoid)
                s_zq = spool.tile([128, 2, MC, BS], f16, tag='s_zq')  # [z, q]
                nc.scalar.activation(s_zq[:], ps_zz[:], AF.Sigmoid)
                m = spool.tile([128, MC, BS], f16, tag='m')
                nc.vector.tensor_tensor(m[:], s_rr[:], ps_tn[:], OP.mult)
                nc.vector.tensor_copy(A[:, :, :, 1], s_zq[:, 1])     # q
                nc.vector.tensor_tensor(Bt[:, :, :, 1], s_zq[:, 0],
                                        hp[:, :, :, 1], OP.mult)     # p = z*h
                nc.scalar.activation(A[:, :, :, 0], m[:], AF.Tanh)
                nc.vector.tensor_tensor_scan(
                    flat(hseq[2 + li][:]), flat(A[:]), flat(Bt[:]), 1.0,
                    OP.mult, OP.add)

            hf = hseq[1 + L]
            nc.sync.dma_start(
                oul[t], hf[:].rearrange('p c b j -> p (c b j)'))
            return hf

        emit_xmms(0)
        hcur, hother = HTa, HTb
        for t in range(Tsteps):
            emit_step(t, hcur, hother)
            hcur, hother = hother, hcur

    nc.compile()
    return nc


def _install_ntff_hook_shim():
    """The agent image lacks ``antenv.axon_hooks``; recreate it and register
    trn_boot's ctypes NTFF hook so trace=True works. Returns True on
    success."""
    import sys
    import types
    try:
        import antenv.axon_hooks  # noqa: F401
        return True
    except ImportError:
        pass
    try:
        import antenv
        from trn_agent_boot.trn_boot import _ntff_profile_via_ctypes
        mod = types.ModuleType('antenv.axon_hooks')
        mod._hook = _ntff_profile_via_ctypes('/opt/axon/libaxon_pjrt.so')
        mod.get_axon_ntff_profile_hook = lambda: mod._hook
        mod.set_axon_ntff_profile_hook = lambda h: setattr(mod, '_hook', h)
        sys.modules['antenv.axon_hooks'] = mod
        antenv.axon_hooks = mod
        return True
    except Exception as e:  # degrade to no-trace
        print(f'ntff hook shim failed: {e}')
        return False


def kernel(x, lengths, Wr, Wz, Wl, Wt, Cx, Ch, Tr, Tz, Tn):
    global LAST_EXEC_NS
    x = np.asarray(x)
    lengths = np.asarray(lengths)

    wp = _pack_weights(Wr, Wz, Wl, Wt, Cx, Ch, Tr, Tz, Tn)
    nc = _build_nc(T)

    in_maps = []
    for k in range(NCORE):
        xs = x[:, k * BS:(k + 1) * BS, :]
        in_maps.append({'xt': _pack_x_shard(xs), 'wp': wp})

    trace = bool(int(os.environ.get('RNN_KERNEL_TRACE', '0')))
    if trace:
        trace = _install_ntff_hook_shim()
    res = run_bass_kernel_spmd(nc, in_maps, core_ids=list(range(NCORE)),
                               trace=trace)
    LAST_EXEC_NS = res.exec_time_ns

    out = np.empty((T, B, H), np.float32)
    for k in range(NCORE):
        y = np.asarray(res.results[k]['out'], np.float32)
        y = y.reshape(T, 128, KC_H, BS, 3)[..., 1].transpose(0, 3, 2, 1)
        out[:, k * BS:(k + 1) * BS, :] = y.reshape(T, BS, H)
    mask = np.arange(T)[:, None] < lengths[None, :]
    out *= mask[:, :, None].astype(np.float32)
    return out
